# revision 15
# baseline (speedup 1.0000x reference)
"""Trainium2 Bass kernel for nn_DeltaSynapse.

I[b,o] = einsum('beo,dbe,deo,dbe->bo', Weff, Xd, delaymap, Wshort+1)
with Weff[b,e,o] = signs[e,o] * (W[e,o]*(1-frac[e,o]) + Wlong[b,e,o]*frac[e,o])

Identity: I[b,o] = sum_e H2[b,e,o] * Weff[b,e,o],
          H2[b,e,o] = sum_d G[d,b,e] * dm[d,e,o],  G = Xd*(Wshort+1).

Shard: 8 o-slices (no=256/core); each core handles all B=16 batches as
two halves of 8.  delaymap is loaded exactly once per core, as fp8-e4m3
(the PE multiplies bf16 stationary x fp8 moving exactly; quantizing dm
costs ~1.4e-2 rel err vs the 2e-2 gate).  HBM/core ~= 21.2MB.

Per block gc (C=8 e-groups of J=16), per b-half hb:
  - gb2[(d,j),(hb,s,j',b)] = G*delta_{j,j'}  (one DVE op per block)
  - H matmuls: Hp[(j',b), o] = gb.T @ dm[:, s-slice]  (4 s per PSUM quad)
  - Z quad = Hp * Weff-tile  (DVE PSUM-direct; ~44% ACT-evac+GpSimd)
  - Zred: I_ps[8, (s%2,o)] += eh.T @ Z  (512-col, accumulated over all
      blocks; PE program order pipelined TWO half-steps back)
DMA: one combined wf DMA per block, dm in 2-block chunks, gpk whole at
start; PE warm-up matmuls fill the initial DMA wait.
Final: DVE tensor_reduce folds the 2 chunks -> [8, no] per half -> out.
"""

import os
import sys
import numpy as np

sys.path.insert(0, "/opt/trn_rl_repo")

import ml_dtypes

BF16 = ml_dtypes.bfloat16
E4M3 = ml_dtypes.float8_e4m3fn

# problem constants
D, B, N = 8, 16, 2048
NCORES = 8
OC = 8            # o-slices (one per core)
NO = N // OC      # per-core o-slice width (256)
J = 16            # e's per group
NG = N // J       # e-groups (128)
HB = B // 2       # b per half (8)
C = 8             # groups per DMA block
NB = NG // C      # DMA blocks (16)


def _consts():
    # eh[p=(j',b), b'] = 1 iff b == b'   (j'-major partitions)
    eh = np.zeros((128, HB), dtype=np.float32)
    p = np.arange(128)
    eh[p, p % HB] = 1.0
    # dmask[p=(d,j), (j', 2*C*HB)] = delta_{j, j'}
    jp = np.arange(J)
    m16 = (p[:, None] % J == jp[None, :]).astype(np.float32)  # [128, 16]
    dmask = np.tile(m16.reshape(128, J, 1), (1, 1, 2 * C * HB))
    return eh, dmask.reshape(128, J * 2 * C * HB)


def host_prep(W, Wlong, Wshort, Xd, delaymap, STDP_frac, signs_pre,
              use_bf16=True, dm_fp8=True):
    """Host-side prep: Weff fusion, packed G, layout transforms, sharding."""
    dt = BF16 if use_bf16 else np.float32
    dt_dm = E4M3 if (use_bf16 and dm_fp8) else dt
    W = np.asarray(W, np.float32)
    frac = np.asarray(STDP_frac, np.float32)
    signs = np.where(W > 0, np.sign(np.asarray(signs_pre, np.float32))[:, None],
                     np.float32(0.0))
    A = signs * W * (1.0 - frac)
    SF = signs * frac
    Weff = (A[None] + SF[None] * np.asarray(Wlong, np.float32))  # [B,N,N] f32
    G = (np.asarray(Xd, np.float32) *
         (np.asarray(Wshort, np.float32) + 1.0))  # [D,B,N]

    # dm[g2, p=(d,j), (blk,s,o)] = delaymap[d, ((2*g2+blk)*C+s)*J+j, oc*NO+o]
    dmf = np.asarray(delaymap, np.float32)
    dm6 = dmf.reshape(D, NB // 2, 2, C, J, N).transpose(1, 0, 4, 2, 3, 5)
    # [NB2, D, J, 2, C, N]

    # wf[gc, p=(j',b), (hb,s,o)] = Weff[hb*HB+b, (gc*C+s)*J+j', oc*NO+o]
    wf6 = Weff.reshape(2, HB, NB, C, J, N).transpose(2, 4, 1, 0, 3, 5)
    # [NB, J, HB, hb, C, N]  -> p=(j',b) j'-major

    # gpk[p=(d,j), gc, (hb,s,b)] = G[d, hb*HB+b, (gc*C+s)*J+j]  (p-major)
    Gr = G.reshape(D, 2, HB, NB, C, J)  # [d,hb,b,gc,s,j]
    gpk_h = Gr.transpose(0, 5, 3, 1, 4, 2)  # [d, j, gc, hb, s, b]

    ins = []
    for core in range(NCORES):
        oc = core
        sl = slice(oc * NO, (oc + 1) * NO)
        ins.append({
            "dm": np.ascontiguousarray(
                dm6[:, :, :, :, :, sl].reshape(NB // 2, 128, 2 * C * NO)).astype(dt_dm),
            "wf": np.ascontiguousarray(
                wf6[:, :, :, :, :, sl].reshape(NB, 128, 2 * C * NO)).astype(dt),
            "gpk": np.ascontiguousarray(
                gpk_h.reshape(128, NB * 2 * C * HB)).astype(dt),
        })
    return ins


def build_nc(use_bf16=True, dm_fp8=True, n_cores=NCORES, no=NO):
    """Build the SPMD Bass program (same on all cores)."""
    import concourse.bass as bass
    import concourse.bacc as bacc
    import concourse.mybir as mybir
    import concourse.tile as tile
    from contextlib import ExitStack

    dt_big = mybir.dt.bfloat16 if use_bf16 else mybir.dt.float32
    dt_dm = mybir.dt.float8e4 if (use_bf16 and dm_fp8) else dt_big
    f32 = mybir.dt.float32
    nb = NB

    nc = bacc.Bacc("TRN2", target_bir_lowering=False, debug=False,
                   num_devices=n_cores)

    dm = nc.declare_dram_parameter("dm", [nb // 2, 128, 2 * C * no], dt_dm,
                                   isOutput=False).ap()
    wf = nc.declare_dram_parameter("wf", [nb, 128, 2 * C * no], dt_big,
                                   isOutput=False).ap()
    gpk = nc.declare_dram_parameter("gpk", [128, nb * 2 * C * HB], dt_big,
                                    isOutput=False).ap()
    out = nc.declare_dram_parameter("out", [B, no], f32, isOutput=True).ap()

    eh_np, dmask_np = _consts()
    np_dt = BF16 if use_bf16 else np.float32
    eh_dram = nc.inline_tensor(eh_np.astype(np_dt), name="ehc")
    dmask_dram = nc.inline_tensor(dmask_np.astype(np_dt), name="dmaskc")

    NQ = C // 4  # Hp quads per step (2)
    ZPIPE = 2    # Zred emitted this many half-steps behind

    with tile.TileContext(nc) as tc, ExitStack() as ctx:
        res = ctx.enter_context(tc.tile_pool(name="res", bufs=1))
        eh_sb = res.tile([128, HB], dt_big)
        nc.sync.dma_start(out=eh_sb[:, :], in_=eh_dram.ap())
        gp_all = res.tile([128, nb * 2 * C * HB], dt_big)
        nc.sync.dma_start(out=gp_all[:, :], in_=gpk)
        dmask_sb = res.tile([128, J * 2 * C * HB], dt_big)
        nc.scalar.dma_start(out=dmask_sb[:, :], in_=dmask_dram.ap())

        hs_pool = ctx.enter_context(tc.tile_pool(name="hsp", bufs=3))
        dm_pool = ctx.enter_context(tc.tile_pool(name="dmp", bufs=3))
        wf_pool = ctx.enter_context(tc.tile_pool(name="wfp", bufs=4))
        gb_pool = ctx.enter_context(tc.tile_pool(name="gbp", bufs=3))
        z_pool = ctx.enter_context(tc.tile_pool(name="zp", bufs=5))
        psum_h = ctx.enter_context(tc.tile_pool(name="psh", bufs=3, space="PSUM"))
        psum_i = ctx.enter_context(tc.tile_pool(name="psi", bufs=1, space="PSUM"))
        out_pool = ctx.enter_context(tc.tile_pool(name="outp", bufs=2))

        # persistent accumulators: [8, (k=2, o)] = 1 PSUM bank per half
        I_ps = [psum_i.tile([HB, 2 * no], f32, name=f"ips{h}", tag=f"ips{h}")
                for h in range(2)]

        # PE warm-up: harmless matmuls during the initial DMA wait keep the
        # HAM clock un-throttled when real work arrives.
        for w in range(16):
            nc.tensor.matmul(I_ps[0][:, :2 * no], eh_sb[:, :],
                             gp_all[:, :2 * no], start=True, stop=True)

        steps = [(gc, hb) for gc in range(nb) for hb in range(2)]
        pend = []  # [(Z_t, hb, gc), ...] awaiting Zred
        gb2 = None
        dm_t = None
        wf_t = None
        qidx = 0  # global quad counter for the DVE/GpSimd split

        def emit_zred(entry):
            pZ, phb, pgc = entry
            for t in range(C // 2):
                nc.tensor.matmul(
                    I_ps[phb][:, :],
                    eh_sb[:, :],
                    pZ[:, 2 * t * no:(2 * t + 2) * no],
                    start=(pgc == 0 and t == 0),
                    stop=(pgc == nb - 1 and t == C // 2 - 1))

        for k, (gc, hb) in enumerate(steps):
            if hb == 0:
                if gc % 2 == 0:
                    dm_t = dm_pool.tile([128, 2 * C * no], dt_dm, tag="dm")
                    nc.sync.dma_start(out=dm_t[:, :], in_=dm[gc // 2])
                wf_t = wf_pool.tile([128, 2 * C * no], dt_big, tag="wf")
                nc.scalar.dma_start(out=wf_t[:, :], in_=wf[gc])
                # expand gb2[p,(hb,s,j',b)] = gpk[p,(hb,s,b)]*delta_{p%16,j'}
                gb2 = gb_pool.tile([128, 2 * C * J * HB], dt_big, tag="gb")
                gslice = gp_all[:, gc * 2 * C * HB:(gc + 1) * 2 * C * HB]
                nc.vector.tensor_mul(
                    gb2.rearrange("p (h s j b) -> p j (h s) b", h=2, s=C, j=J),
                    gslice.rearrange("p (hs b) -> p hs b", b=HB)
                          .unsqueeze(1).broadcast_to((128, J, 2 * C, HB)),
                    dmask_sb.rearrange("p (j hs b) -> p j hs b", j=J, b=HB))

            gb_v = gb2.rearrange("p (h s m) -> p h s m", h=2, s=C)
            Z_t = z_pool.tile([128, C * no], dt_big, tag="z")
            dmo = (gc % 2) * C * no
            wfo = hb * C * no

            hp_tiles = []
            for t in range(NQ):
                Hp = psum_h.tile([128, 4 * no], f32, tag="hp")
                for i in range(4):
                    s = 4 * t + i
                    nc.tensor.matmul(Hp[:, i * no:(i + 1) * no],
                                     gb_v[:, hb, s, :],
                                     dm_t[:, dmo + s * no:dmo + (s + 1) * no],
                                     start=True, stop=True)
                hp_tiles.append(Hp)

            # elementwise Z = wf * Hp  (quad tiles of 1024)
            for t in range(NQ):
                Hp = hp_tiles[t]
                so = slice(4 * t * no, (4 * t + 4) * no)
                wso = slice(wfo + 4 * t * no, wfo + (4 * t + 4) * no)
                # ~44% of quads go via ACT evac + GpSimd, rest DVE direct
                on_gs = (qidx % 9) in (0, 2, 4, 6)
                qidx += 1
                if on_gs:
                    Hs = hs_pool.tile([128, 4 * no], dt_big, tag="hs")
                    nc.scalar.copy(Hs[:, :], Hp[:, :])
                    nc.gpsimd.tensor_mul(Z_t[:, so], wf_t[:, wso], Hs[:, :])
                else:
                    nc.vector.tensor_mul(Z_t[:, so], wf_t[:, wso], Hp[:, :])

            # Zred for an OLDER step (software pipeline, keeps PE fed)
            pend.append((Z_t, hb, gc))
            if len(pend) > ZPIPE:
                emit_zred(pend.pop(0))

        for entry in pend:
            emit_zred(entry)

        # fold chunks: [8, (k,o)] viewed as [8, o, k] -> reduce X
        for hb in range(2):
            I_sb = out_pool.tile([HB, no], f32, name=f"isb{hb}", tag="isb")
            nc.vector.tensor_reduce(I_sb[:, :],
                                    I_ps[hb].rearrange("b (k o) -> b o k", k=2),
                                    axis=mybir.AxisListType.X,
                                    op=mybir.AluOpType.add)
            nc.sync.dma_start(out=out[hb * HB:(hb + 1) * HB, :], in_=I_sb[:, :])

    nc.compile()
    return nc


_CACHE = {}


def kernel(W, Wlong, Wshort, Xd, delaymap, STDP_frac, signs_pre):
    from concourse.bass_utils import run_bass_kernel_spmd

    use_bf16 = os.environ.get("DS_FP32", "0") != "1"
    dm_fp8 = os.environ.get("DS_DM8", "1") == "1"
    ins = host_prep(W, Wlong, Wshort, Xd, delaymap, STDP_frac, signs_pre,
                    use_bf16, dm_fp8)
    key = ("nc", use_bf16, dm_fp8)
    if key not in _CACHE:
        _CACHE[key] = build_nc(use_bf16, dm_fp8)
    nc = _CACHE[key]
    r = run_bass_kernel_spmd(nc, ins, list(range(NCORES)))
    out_full = np.zeros((B, N), np.float32)
    for core in range(NCORES):
        oc = core
        out_full[:, oc * NO:(oc + 1) * NO] = \
            r.results[core]["out"].astype(np.float32)
    return out_full


if __name__ == "__main__":
    pass
